# revision 21
# baseline (speedup 1.0000x reference)
"""TRN2 Bass kernel for nn_CutsSelector (GNN message passing).

Strategy (node-parallel over destination windows, 8 cores):
  By linearity of the g-Linear, seg_sum(msg)[n] =
      cnt[n]*(x[n] @ g_w_dst) + (seg_sum_{dst=n} x[src]) @ g_w_src
      + s_attr[n]*g_w_e + cnt[n]*g_b
  so the only per-edge device work is X_agg[n] = sum_{e: dst=n} x[src[e]].
  cnt/s_attr (scalar segment sums over edge_index/edge_attr) are computed on
  host during packing.

  The HW gather is descriptor-rate bound (~5ns/desc/core) with a secondary
  sustained byte limit, so the host planner emits MIXED descriptors per
  window over the dst-sorted, then src-sorted edge list:
    - pair descs (1KB): two edges whose srcs are consecutive rows (s, s+1)
      share one descriptor into a 2-row span table x_pair[r] = rows [r, r+2);
    - single descs (512B): everything else, from the plain hi/lo row table.
  This cuts descriptors ~20% vs one-per-edge at unchanged total bytes.
  Per 128-desc block the aggregation runs one one-hot scatter matmul per
  row-quarter (S = is_equal(iota, dstrel col) on DVE;
  psum[d,0:256] += S^T @ gt[...] on PE, hi|lo as 256 cols). The epilogue
  applies the g/f/classifier linears per 128-dst window (exact fp32 matmuls)
  and a sigmoid; hi+lo halves are summed on DVE during PSUM evacuation.

kernel(**inputs) takes the FULL unsharded inputs and returns (y, probs).
"""

import sys

if "/opt/trn_rl_repo" not in sys.path:
    sys.path.insert(0, "/opt/trn_rl_repo")

import numpy as np
import ml_dtypes

import concourse.bacc as bacc
import concourse.mybir as mybir
from concourse import tile
from concourse.bass_utils import run_bass_kernel_spmd

F32 = mybir.dt.float32
BF16 = mybir.dt.bfloat16
I16 = mybir.dt.int16
U8 = mybir.dt.uint8
AL = mybir.AluOpType

C = 128
N_CORES = 8


def build_kernel(NW, BS, NPAD, n_cores=N_CORES, repeat=1, mode="full", gch=8,
                 nq=4, scratch=16384):
    # mode: "full" | "nogather" | "noscatter"  [ablation timing only]
    BS1, BS2, V1, V2 = BS
    NB1 = sum(BS1)
    NB2 = sum(BS2)
    N_OWN = NW * 128

    nc = bacc.Bacc("TRN2", target_bir_lowering=False, debug=False,
                   num_devices=n_cores, num_swdge_queues=nq,
                   dynamic_dma_scratch_size=scratch)

    x_sing = nc.dram_tensor("x_sing", [NPAD, 2 * C], BF16, kind="ExternalInput")
    x_pair = nc.dram_tensor("x_pair", [NPAD, 4 * C], BF16, kind="ExternalInput")
    xT_own = nc.dram_tensor("xT_own", [C, N_OWN], F32, kind="ExternalInput")
    idx1 = nc.dram_tensor("idx1", [128, NB1 * 8], I16, kind="ExternalInput")
    idx2 = nc.dram_tensor("idx2", [128, NB2 * 8], I16, kind="ExternalInput")
    dr1_d = nc.dram_tensor("dr1", [128, NB1], F32, kind="ExternalInput")
    dr2_d = nc.dram_tensor("dr2", [128, NB2 * 2], F32, kind="ExternalInput")
    # per-node host-precomputed stats: [128, NW*4] = (rcol, ind, smean, unused)
    nstat = nc.dram_tensor("nstat", [128, NW * 4], F32, kind="ExternalInput")
    gw_dst = nc.dram_tensor("gw_dst", [C, C], F32, kind="ExternalInput")
    gw_src = nc.dram_tensor("gw_src", [C, C], F32, kind="ExternalInput")
    gwe_row = nc.dram_tensor("gwe_row", [1, C], F32, kind="ExternalInput")
    gb_row = nc.dram_tensor("gb_row", [1, C], F32, kind="ExternalInput")
    fw1 = nc.dram_tensor("fw1", [C, C], F32, kind="ExternalInput")
    fw2 = nc.dram_tensor("fw2", [C, C], F32, kind="ExternalInput")
    fb_col = nc.dram_tensor("fb_col", [C, 1], F32, kind="ExternalInput")
    clsw = nc.dram_tensor("clsw", [C, 1], F32, kind="ExternalInput")
    clsb = nc.dram_tensor("clsb", [1, 1], F32, kind="ExternalInput")
    probs_out = nc.dram_tensor("probs_out", [1, N_OWN], F32, kind="ExternalOutput")
    y_out = nc.dram_tensor("y_out", [1, N_OWN], U8, kind="ExternalOutput")

    with tile.TileContext(nc) as tc:
        with (
            tc.tile_pool(name="persist", bufs=1) as pp,
            tc.tile_pool(name="g1", bufs=4) as gp1,
            tc.tile_pool(name="g2", bufs=4) as gp2,
            tc.tile_pool(name="sbloop", bufs=2) as sp,
            tc.tile_pool(name="stile", bufs=6) as stp,
            tc.tile_pool(name="pacc", bufs=3, space="PSUM") as pacc,
            tc.tile_pool(name="ptmp", bufs=3, space="PSUM") as ptmp,
            tc.tile_pool(name="pcls", bufs=2, space="PSUM") as pcls,
        ):
            t_xT = pp.tile([C, N_OWN], F32, tag="xT")
            nc.sync.dma_start(t_xT[:], xT_own[:])
            t_i1 = pp.tile([128, NB1 * 8], I16, tag="i1")
            nc.sync.dma_start(t_i1[:], idx1[:])
            t_i2 = pp.tile([128, NB2 * 8], I16, tag="i2")
            nc.sync.dma_start(t_i2[:], idx2[:])
            t_d1 = pp.tile([128, NB1], F32, tag="d1")
            nc.sync.dma_start(t_d1[:], dr1_d[:])
            t_d2 = pp.tile([128, NB2 * 2], F32, tag="d2")
            nc.sync.dma_start(t_d2[:], dr2_d[:])
            t_ns = pp.tile([128, NW * 4], F32, tag="nstat")
            nc.sync.dma_start(t_ns[:], nstat[:])
            t_gwd = pp.tile([C, C], F32, tag="gwd")
            nc.sync.dma_start(t_gwd[:], gw_dst[:])
            t_gws = pp.tile([C, C], F32, tag="gws")
            nc.sync.dma_start(t_gws[:], gw_src[:])
            t_gwe = pp.tile([1, C], F32, tag="gwe")
            nc.sync.dma_start(t_gwe[:], gwe_row[:])
            t_gb = pp.tile([1, C], F32, tag="gb")
            nc.sync.dma_start(t_gb[:], gb_row[:])
            t_fw1 = pp.tile([C, C], F32, tag="fw1")
            nc.sync.dma_start(t_fw1[:], fw1[:])
            t_fw2 = pp.tile([C, C], F32, tag="fw2")
            nc.sync.dma_start(t_fw2[:], fw2[:])
            t_fb = pp.tile([C, 1], F32, tag="fb")
            nc.sync.dma_start(t_fb[:], fb_col[:])
            t_clsw = pp.tile([C, 1], F32, tag="clsw")
            nc.sync.dma_start(t_clsw[:], clsw[:])
            t_clsb = pp.tile([1, 1], F32, tag="clsb")
            nc.sync.dma_start(t_clsb[:], clsb[:])

            t_iota = pp.tile([128, 128], BF16, tag="iota")
            nc.gpsimd.iota(t_iota[:], pattern=[[1, 128]], base=0,
                           channel_multiplier=0,
                           allow_small_or_imprecise_dtypes=True)
            t_iotac = pp.tile([128, 1], F32, tag="iotac")
            nc.gpsimd.iota(t_iotac[:], pattern=[[1, 1]], base=0,
                           channel_multiplier=1,
                           allow_small_or_imprecise_dtypes=True)
            t_ident = pp.tile([128, 128], F32, tag="ident")
            nc.vector.tensor_scalar(t_ident[:], t_iota[:], t_iotac[:], None,
                                    AL.is_equal)
            t_ones1 = pp.tile([1, 128], F32, tag="ones1")
            nc.vector.memset(t_ones1[:], 1.0)
            ps_b = ptmp.tile([128, C], F32, tag="ptmp")
            nc.tensor.matmul(ps_b[:], t_ones1[:], t_gb[:], start=True, stop=True)
            t_GB = pp.tile([128, C], F32, tag="GB")
            nc.vector.tensor_copy(t_GB[:], ps_b[:])
            ps_b2 = ptmp.tile([128, C], F32, tag="ptmp")
            nc.tensor.matmul(ps_b2[:], t_ones1[:], t_gwe[:], start=True, stop=True)
            t_GWE = pp.tile([128, C], F32, tag="GWE")
            nc.vector.tensor_copy(t_GWE[:], ps_b2[:])

            t_z = pp.tile([1, N_OWN], F32, tag="zrow")

            gq = 0
            for _rep in range(repeat):
              b1o = 0
              b2o = 0
              for w in range(NW):
                B1 = BS1[w]
                B2 = BS2[w]
                gt1 = gp1.tile([128, B1, 2 * C], BF16, tag="gt1")
                gt2 = gp2.tile([128, B2, 4 * C], BF16, tag="gt2")
                pb1 = V1[w] // 128
                if pb1 < B1:
                    nc.vector.memset(gt1[:, pb1:B1, :], 0.0)
                pb2 = V2[w] // 128
                if pb2 < B2:
                    nc.vector.memset(gt2[:, pb2:B2, :], 0.0)
                if mode == "nogather":
                    r0 = (b1o * 128) % max(1, NPAD - B1 * 128)
                    nc.sync.dma_start(
                        gt1[:], x_sing.ap()[r0:r0 + B1 * 128, :]
                        .rearrange("(p b) c -> p b c", p=128))
                    r2 = (b2o * 128) % max(1, NPAD - B2 * 128)
                    nc.sync.dma_start(
                        gt2[:], x_pair.ap()[r2:r2 + B2 * 128, :]
                        .rearrange("(p b) c -> p b c", p=128))
                else:
                    for g0 in range(0, B1, gch):
                        g1 = min(B1, g0 + gch)
                        ni = min((g1 - g0) * 128, V1[w] - g0 * 128)
                        if ni <= 0:
                            continue
                        gb = -(-ni // 128)
                        nc.gpsimd.dma_gather(
                            gt1[:, g0:g0 + gb, :], x_sing[:],
                            t_i1[:, (b1o + g0) * 8:
                                  (b1o + g0) * 8 + -(-ni // 16)],
                            ni, ni, 2 * C, queue_num=gq % nq)
                        gq += 1
                    for g0 in range(0, B2, gch):
                        g1 = min(B2, g0 + gch)
                        ni = min((g1 - g0) * 128, V2[w] - g0 * 128)
                        if ni <= 0:
                            continue
                        gb = -(-ni // 128)
                        nc.gpsimd.dma_gather(
                            gt2[:, g0:g0 + gb, :], x_pair[:],
                            t_i2[:, (b2o + g0) * 8:
                                  (b2o + g0) * 8 + -(-ni // 16)],
                            ni, ni, 4 * C, queue_num=gq % nq)
                        gq += 1

                ps = pacc.tile([128, 256], F32, tag="pacc")
                if mode == "noscatter":
                    nc.vector.memset(ps[:], 0.0)
                    sink = sp.tile([128, 4], BF16, tag="sink")
                    nc.vector.tensor_copy(sink[:, 0:2], gt1[:, 0, 0:2])
                    nc.vector.tensor_copy(sink[:, 2:4], gt2[:, 0, 0:2])
                else:
                    for b in range(B1):
                        col = b1o + b
                        S = stp.tile([128, 128], BF16, tag="S")
                        nc.vector.tensor_scalar(
                            S[:], t_iota[:], t_d1[:, col:col + 1], None,
                            AL.is_equal)
                        nc.tensor.matmul(ps[:], S[:], gt1[:, b, :],
                                         start=(b == 0), stop=False)
                    for b in range(B2):
                        col = b2o + b
                        for q in range(2):
                            S = stp.tile([128, 128], BF16, tag="S")
                            nc.vector.tensor_scalar(
                                S[:], t_iota[:],
                                t_d2[:, col * 2 + q:col * 2 + q + 1], None,
                                AL.is_equal)
                            nc.tensor.matmul(
                                ps[:], S[:], gt2[:, b, q * 256:(q + 1) * 256],
                                start=False,
                                stop=(b == B2 - 1 and q == 1))

                # fold hi+lo, apply node stats
                xlo = sp.tile([128, 128], F32, tag="xlo")
                nc.vector.tensor_copy(xlo[:], ps[:, 128:256])
                xa = sp.tile([128, 128], F32, tag="xa")
                nc.vector.tensor_add(xa[:], ps[:, 0:128], xlo[:])
                rcol = sp.tile([128, 1], F32, tag="rcol")
                nc.vector.tensor_copy(rcol[:], t_ns[:, 4 * w:4 * w + 1])
                ind = sp.tile([128, 1], F32, tag="ind")
                nc.vector.tensor_copy(ind[:], t_ns[:, 4 * w + 1:4 * w + 2])
                sm = sp.tile([128, 1], F32, tag="sm")
                nc.vector.tensor_copy(sm[:], t_ns[:, 4 * w + 2:4 * w + 3])

                pst = ptmp.tile([128, 128], F32, tag="ptmp")
                nc.tensor.transpose(pst[:], xa[:], t_ident[:])
                xaT = sp.tile([128, 128], F32, tag="xaT")
                nc.vector.tensor_copy(xaT[:], pst[:])

                ps2 = ptmp.tile([128, 128], F32, tag="ptmp")
                nc.tensor.matmul(ps2[:], xaT[:], t_gws[:], start=True, stop=True)
                ps3 = ptmp.tile([128, 128], F32, tag="ptmp")
                nc.tensor.matmul(ps3[:], t_xT[:, w * 128:(w + 1) * 128],
                                 t_gwd[:], start=True, stop=True)

                a1 = sp.tile([128, 128], F32, tag="a1")
                nc.vector.tensor_scalar(a1[:], ps2[:], rcol[:], None, AL.mult)
                tt = sp.tile([128, 128], F32, tag="tt")
                nc.vector.tensor_add(tt[:], ps3[:], t_GB[:])
                tt2 = sp.tile([128, 128], F32, tag="tt2")
                nc.vector.tensor_scalar(tt2[:], tt[:], ind[:], None, AL.mult)
                a2 = sp.tile([128, 128], F32, tag="a2")
                nc.vector.tensor_add(a2[:], a1[:], tt2[:])
                t3 = sp.tile([128, 128], F32, tag="t3")
                nc.vector.tensor_scalar(t3[:], t_GWE[:], sm[:], None, AL.mult)
                aggr = sp.tile([128, 128], F32, tag="aggr")
                nc.vector.tensor_add(aggr[:], a2[:], t3[:])

                pst2 = ptmp.tile([128, 128], F32, tag="ptmp")
                nc.tensor.transpose(pst2[:], aggr[:], t_ident[:])
                agT = sp.tile([128, 128], F32, tag="agT")
                nc.vector.tensor_copy(agT[:], pst2[:])

                ps4 = ptmp.tile([128, 128], F32, tag="ptmp")
                nc.tensor.matmul(ps4[:], t_fw1[:], t_xT[:, w * 128:(w + 1) * 128],
                                 start=True, stop=False)
                nc.tensor.matmul(ps4[:], t_fw2[:], agT[:], start=False, stop=True)
                hT = sp.tile([128, 128], F32, tag="hT")
                nc.vector.tensor_scalar_add(hT[:], ps4[:], t_fb[:])

                ps5 = pcls.tile([1, 128], F32, tag="pcls")
                nc.tensor.matmul(ps5[:], t_clsw[:], hT[:], start=True, stop=True)
                nc.vector.tensor_copy(t_z[0:1, w * 128:(w + 1) * 128], ps5[:])

                b1o += B1
                b2o += B2

            zb = pp.tile([1, N_OWN], F32, tag="zb")
            nc.vector.tensor_scalar_add(zb[:], t_z[:], t_clsb[:])
            pr = pp.tile([1, N_OWN], F32, tag="pr")
            nc.scalar.activation(pr[:], zb[:],
                                 mybir.ActivationFunctionType.Sigmoid)
            yr = pp.tile([1, N_OWN], U8, tag="yr")
            nc.vector.tensor_scalar(yr[:], zb[:], 0.0, None, AL.is_gt)
            nc.sync.dma_start(probs_out[:], pr[:])
            nc.sync.dma_start(y_out[:], yr[:])

    nc.compile()
    return nc


def _plan_window(srcs, dsts_rel):
    """Mixed descriptor planning for one window (srcs sorted ascending).

    Pair desc: two edges at consecutive rows (s, s+1). Single desc: one edge.
    Returns (st1, d1, st2, d2[:, 2])."""
    order = np.argsort(srcs, kind="stable")
    s = srcs[order]
    d = dsts_rel[order]
    st1, d1, st2, d2 = [], [], [], []
    i = 0
    N = len(s)
    while i < N:
        if i + 1 < N and s[i + 1] == s[i] + 1:
            st2.append(int(s[i]))
            d2.append((float(d[i]), float(d[i + 1])))
            i += 2
        else:
            st1.append(int(s[i]))
            d1.append(float(d[i]))
            i += 1
    return (np.asarray(st1, np.int64), np.asarray(d1, np.float32),
            np.asarray(st2, np.int64), np.asarray(d2, np.float32).reshape(-1, 2))


def pack_inputs(x_a, edge_index, edge_attr, g_w, g_b, f_w, f_b, cls_w, cls_b,
                n_cores=N_CORES):
    N = x_a.shape[0]
    NW_TOT = -(-N // 128)
    NW_TOT = -(-NW_TOT // n_cores) * n_cores
    NPAD = NW_TOT * 128
    NW = NW_TOT // n_cores

    src = np.asarray(edge_index[0], dtype=np.int64)
    dst = np.asarray(edge_index[1], dtype=np.int64)
    attr = np.asarray(edge_attr[:, 0], dtype=np.float32)

    order = np.argsort(dst, kind="stable")
    dst_s = dst[order]
    src_s = src[order]
    win = dst_s // 128

    wstart = np.searchsorted(win, np.arange(NW_TOT))
    wend = np.searchsorted(win, np.arange(NW_TOT) + 1)

    # host per-node stats
    cnt = np.bincount(dst, minlength=NPAD).astype(np.float64)
    s_attr = np.zeros(NPAD, np.float64)
    np.add.at(s_attr, dst, attr.astype(np.float64))
    rcol_n = (1.0 / np.maximum(cnt, 1.0)).astype(np.float32)
    ind_n = (cnt >= 1.0).astype(np.float32)
    sm_n = (s_attr / np.maximum(cnt, 1.0)).astype(np.float32)

    plans = {}
    BS1 = [1] * NW
    BS2 = [1] * NW
    V1 = [1] * NW
    V2 = [1] * NW
    for c in range(n_cores):
        for s_ in range(NW):
            g = c * NW + s_
            e0, e1 = int(wstart[g]), int(wend[g])
            p = _plan_window(src_s[e0:e1],
                             (dst_s[e0:e1] - g * 128).astype(np.float32))
            plans[(c, s_)] = p
            BS1[s_] = max(BS1[s_], -(-max(1, len(p[0])) // 128))
            BS2[s_] = max(BS2[s_], -(-max(1, len(p[2])) // 128))
            V1[s_] = max(V1[s_], max(1, len(p[0])))
            V2[s_] = max(V2[s_], max(1, len(p[2])))
    NB1 = sum(BS1)
    NB2 = sum(BS2)

    x_pad = np.zeros((NPAD + 2, C), np.float32)
    x_pad[:N] = np.asarray(x_a, np.float32)
    x_hi = x_pad.astype(ml_dtypes.bfloat16)
    x_lo = (x_pad - x_hi.astype(np.float32)).astype(ml_dtypes.bfloat16)
    x_hilo = np.concatenate([x_hi, x_lo], axis=1)  # [NPAD+2, 2C]
    x_sing = np.ascontiguousarray(x_hilo[:NPAD])
    x_pair = np.ascontiguousarray(
        np.concatenate([x_hilo[0:NPAD], x_hilo[1:NPAD + 1]], axis=1))

    gw = np.asarray(g_w, np.float32)
    fw = np.asarray(f_w, np.float32)
    shared = {
        "x_sing": x_sing,
        "x_pair": x_pair,
        "gw_dst": gw[0:C],
        "gw_src": gw[C:2 * C],
        "gwe_row": gw[2 * C:2 * C + 1],
        "gb_row": np.asarray(g_b, np.float32).reshape(1, C),
        "fw1": fw[0:C],
        "fw2": fw[C:2 * C],
        "fb_col": np.asarray(f_b, np.float32).reshape(C, 1),
        "clsw": np.asarray(cls_w, np.float32).reshape(C, 1),
        "clsb": np.asarray(cls_b, np.float32).reshape(1, 1),
    }

    def wrap_idx(starts, nblocks):
        nd = nblocks * 128
        stp_ = np.zeros(nd, np.int64)
        stp_[:len(starts)] = starts
        return np.tile(stp_.reshape(nblocks * 8, 16).T.astype(np.int16), (8, 1))

    in_maps = []
    for c in range(n_cores):
        si1 = np.zeros((128, NB1 * 8), np.int16)
        si2 = np.zeros((128, NB2 * 8), np.int16)
        dr1 = np.full((128, NB1), -1.0, np.float32)
        dr2 = np.full((128, NB2 * 2), -1.0, np.float32)
        nstat = np.zeros((128, NW * 4), np.float32)
        b1o = b2o = 0
        for s_ in range(NW):
            g = c * NW + s_
            st1, d1, st2, d2 = plans[(c, s_)]
            B1, B2 = BS1[s_], BS2[s_]
            si1[:, b1o * 8:(b1o + B1) * 8] = wrap_idx(st1, B1)
            si2[:, b2o * 8:(b2o + B2) * 8] = wrap_idx(st2, B2)
            dp1 = np.full(B1 * 128, -1.0, np.float32)
            dp1[:len(d1)] = d1
            dr1[:, b1o:b1o + B1] = dp1.reshape(B1, 128).T
            dp2 = np.full((B2 * 128, 2), -1.0, np.float32)
            if len(st2):
                dp2[:len(st2)] = d2
            dr2[:, b2o * 2:(b2o + B2) * 2] = \
                dp2.reshape(B2, 128, 2).transpose(1, 0, 2).reshape(128, B2 * 2)
            nstat[:, 4 * s_ + 0] = rcol_n[g * 128:(g + 1) * 128]
            nstat[:, 4 * s_ + 1] = ind_n[g * 128:(g + 1) * 128]
            nstat[:, 4 * s_ + 2] = sm_n[g * 128:(g + 1) * 128]
            b1o += B1
            b2o += B2
        m = dict(shared)
        m["xT_own"] = np.ascontiguousarray(
            x_pad[c * NW * 128:(c + 1) * NW * 128].T)
        m["idx1"] = si1
        m["idx2"] = si2
        m["dr1"] = dr1
        m["dr2"] = dr2
        m["nstat"] = nstat
        in_maps.append(m)

    meta = dict(NW=NW, BS=(tuple(BS1), tuple(BS2), tuple(V1), tuple(V2)),
                NPAD=NPAD, N=N, n_cores=n_cores)
    return in_maps, meta


_NC_CACHE = {}


def run(inputs: dict, trace: bool = False, trace_kwargs=None):
    """Pack, build (cached), execute on 8 cores. Returns ((y, probs), results)."""
    in_maps, meta = pack_inputs(**inputs)
    key = (meta["NW"], meta["BS"], meta["NPAD"], meta["n_cores"])
    nc = _NC_CACHE.get(key)
    if nc is None:
        nc = build_kernel(meta["NW"], meta["BS"], meta["NPAD"], meta["n_cores"])
        _NC_CACHE[key] = nc
    res = run_bass_kernel_spmd(nc, in_maps, list(range(meta["n_cores"])),
                               trace=trace, **(trace_kwargs or {}))
    N = meta["N"]
    probs = np.concatenate([r["probs_out"].reshape(-1) for r in res.results])[:N]
    y = np.concatenate([r["y_out"].reshape(-1) for r in res.results])[:N]
    out = (y.astype(bool).reshape(N, 1), probs.reshape(N, 1).astype(np.float32))
    return out, res


def kernel(**inputs):
    out, _ = run(inputs, trace=False)
    return out
